# revision 24
# baseline (speedup 1.0000x reference)
"""Trainium2 Bass kernel for nn_DecNP (two-stage KNN feature propagation).

Algorithm (per stage): for each query point, find K=8 nearest coarse points
(PE matmul for ranking + DVE max8/find_index8), gather their packed rows
(xyz | percentages | normalized directions | features) via indirect DMA,
compute direction-mask weights on DVE, interpolate features via PE
diagonal-weight matmuls, then combine with the skip connection and
L2-normalize.  Stage 0: 1024 -> 4096 points, stage 1: 4096 -> 16384.

Sharding: query rows split across 8 cores.  Stage-0 output is AllGather'd
(it is the gather table of stage 1); the scalar mean of de_k_weight_sum is
AllReduce'd per stage.

Emission order is tuned for the in-order engine queues: stage-1's
distance/top-k work (part A) is emitted so it fills the engines while
stage 0 finishes and the collectives run; gather/interp (part B) follows.
"""
import sys

for _p in ("/opt/trn_rl_repo", "/root/.axon_site/_ro/trn_rl_repo", "/root/.axon_site"):
    if _p not in sys.path:
        sys.path.append(_p)

import numpy as np

import concourse.bacc as bacc
import concourse.bass as bass
import concourse.bass_isa as bass_isa
import concourse.mybir as mybir
from concourse.masks import make_identity
from concourse.tile import TileContext

NCORES = 8
P = 128
D = 768
K = 8
M = 20
GAMMA = 0.85
EPS_DIR = 1e-8
TW = 84 + D    # fp32 host-packed table row: 0:3 xyz | 3:23 perc | 23:83 dirs | pad | 84: feat
METAB = 168    # bf16 columns holding the 84 fp32 meta words (bitcast)
TWB = METAB + D  # bf16 gather-table row
BF16 = mybir.dt.bfloat16
F32 = mybir.dt.float32
X = mybir.AxisListType.X
Copy = mybir.ActivationFunctionType.Copy
Sqrt = mybir.ActivationFunctionType.Sqrt
Square = mybir.ActivationFunctionType.Square
Abs = mybir.ActivationFunctionType.Abs

ST0 = dict(S=1024, Q=512, NT=4096)
ST1 = dict(S=4096, Q=2048, NT=16384)
C_SCAL = 0.3  # N == 4*S in both stages

RG = [list(range(NCORES))]

_CACHE = {}


class Stage:
    def __init__(self, nc, pools, ident, *, st, S, Q, NT, tp_src, tw, qxyz, p1,
                 f1d, out_rows, sum_in, sum_out, fill_feat, out_bf):
        self.__dict__.update(locals())
        self.n_st = S // P
        self.n_qt = Q // P
        self.tpa = tp_src.ap()
        self.twa = tw.ap()
        self.p1a = p1.ap()
        self.f1da = f1d.ap()
        self.ora = out_rows.ap()

    def emit_tables(self):
        nc, pools, ident = self.nc, self.pools, self.ident
        st, S, Q = self.st, self.S, self.Q
        tpa, twa = self.tpa, self.twa
        # meta words bitcast fp32 -> raw bf16 pairs
        nc.sync.dma_start(out=twa[:, 0:METAB], in_=tpa[:, 0:84].bitcast(BF16))
        if self.fill_feat:
            for i in range(self.n_st):
                rs = slice(i * P, (i + 1) * P)
                ft = pools["f1"].tile([P, D], F32, tag="ftc")
                nc.sync.dma_start(out=ft[:, :], in_=tpa[rs, 84:84 + D])
                fb = pools["f1"].tile([P, D], BF16, tag="ftb")
                nc.scalar.activation(out=fb[:, :], in_=ft[:, :], func=Copy)
                nc.sync.dma_start(out=twa[rs, METAB:TWB], in_=fb[:, :])

        # coarse table C4 rows: (x, y, z, |s|^2), transposed; dirs normalized
        c4 = pools["tbl"].tile([4, S], F32, tag=f"c4_{st}")
        self.c4 = c4
        for i in range(self.n_st):
            rs = slice(i * P, (i + 1) * P)
            t84 = pools["work"].tile([P, 84], F32, tag="t84")
            nc.sync.dma_start(out=t84[:, :], in_=tpa[rs, 0:84])
            ca = pools["work"].tile([P, 4], F32, tag="ca")
            nc.vector.tensor_copy(ca[:, 0:3], t84[:, 0:3])
            sq3 = pools["work"].tile([P, 3], F32, tag="sq3")
            nc.vector.tensor_mul(sq3[:, :], t84[:, 0:3], t84[:, 0:3])
            nc.vector.reduce_sum(out=ca[:, 3:4], in_=sq3[:, :], axis=X)
            ptp = pools["pt"].tile([4, P], F32, tag="ptp")
            nc.tensor.transpose(out=ptp[:, :], in_=ca[:, :], identity=ident[:, :])
            nc.scalar.activation(out=c4[:, rs], in_=ptp[:, :], func=Copy)
            dsq = pools["work"].tile([P, 60], F32, tag="dsq")
            nc.vector.tensor_mul(dsq[:, :], t84[:, 23:83], t84[:, 23:83])
            n2 = pools["work"].tile([P, M], F32, tag="n2")
            nc.vector.reduce_sum(out=n2[:, :],
                                 in_=dsq[:, :].rearrange("p (m c) -> p m c", c=3), axis=X)
            nrm = pools["work"].tile([P, M], F32, tag="nrm")
            nc.scalar.activation(out=nrm[:, :], in_=n2[:, :], func=Sqrt)
            nc.vector.tensor_scalar_add(nrm[:, :], nrm[:, :], EPS_DIR)
            rin = pools["work"].tile([P, M], F32, tag="rin")
            nc.vector.reciprocal(rin[:, :], nrm[:, :])
            dn = pools["work"].tile([P, 60], F32, tag="dn")
            nc.vector.tensor_mul(
                dn[:, :].rearrange("p (m c) -> p m c", c=3),
                t84[:, 23:83].rearrange("p (m c) -> p m c", c=3),
                rin[:, :].unsqueeze(2).to_broadcast([P, M, 3]),
            )
            nc.sync.dma_start(out=twa[rs, 46:166], in_=dn[:, :].bitcast(BF16))

        # query coords (kept resident) and Q4 = (2x, 2y, 2z, -1)^T
        qxall = pools["tbl"].tile([P, self.n_qt, 3], F32, tag=f"qxall_{st}")
        self.qxall = qxall
        q4 = pools["tbl"].tile([4, Q], F32, tag=f"q4_{st}")
        self.q4 = q4
        qxa = self.qxyz.ap()
        for t in range(self.n_qt):
            rs = slice(t * P, (t + 1) * P)
            nc.sync.dma_start(out=qxall[:, t, :], in_=qxa[rs, :])
            qt4 = pools["work"].tile([P, 4], F32, tag="qt4")
            nc.vector.tensor_scalar_mul(qt4[:, 0:3], qxall[:, t, :], 2.0)
            nc.vector.memset(qt4[:, 3:4], -1.0)
            ptp = pools["pt"].tile([4, P], F32, tag="ptp")
            nc.tensor.transpose(out=ptp[:, :], in_=qt4[:, :], identity=ident[:, :])
            nc.scalar.activation(out=q4[:, rs], in_=ptp[:, :], func=Copy)

        # per-tile top-k indices, kept resident between part A and part B
        self.idxall = pools["tbl"].tile([P, self.n_qt, K], mybir.dt.uint32,
                                        tag=f"idxall_{st}")
        self.acc = pools["tbl"].tile([P, 1], F32, tag=f"acc_{st}")
        nc.vector.memset(self.acc[:, :], 0.0)
        # stage 0 is small enough to keep interp results in SBUF
        self.f1keep = None
        if self.Q <= 512:
            self.f1keep = pools["tbl"].tile([P, self.n_qt, D], F32,
                                            tag=f"f1keep_{st}")

    def emit_part_a(self, tiles):
        """Distance ranking + top-8 indices for the given tile range."""
        nc, pools = self.nc, self.pools
        S = self.S
        for t in tiles:
            rs = slice(t * P, (t + 1) * P)
            negE = pools["neg"].tile([P, S], F32, tag="negE")
            for c in range(S // 512):
                pe = pools["pe"].tile([P, 512], F32, tag="pe")
                nc.tensor.matmul(out=pe[:, :], lhsT=self.q4[:, rs],
                                 rhs=self.c4[:, c * 512:(c + 1) * 512],
                                 start=True, stop=True)
                nc.scalar.activation(out=negE[:, c * 512:(c + 1) * 512],
                                     in_=pe[:, :], func=Copy)
            best = pools["work"].tile([P, K], F32, tag="best")
            nc.vector.max(out=best[:, :], in_=negE[:, :])
            nc.vector.max_index(out=self.idxall[:, t, :], in_max=best[:, :],
                                in_values=negE[:, :])

    def emit_part_b(self, tiles):
        """Gather neighbours, weights, interpolation for the given tiles."""
        nc, pools = self.nc, self.pools
        twa = self.twa
        for t in tiles:
            gt = pools["gt"].tile([P, K, TWB], BF16, tag="gt")
            for k in range(K):
                nc.gpsimd.indirect_dma_start(
                    out=gt[:, k, :], out_offset=None,
                    in_=twa[:, :],
                    in_offset=bass.IndirectOffsetOnAxis(
                        ap=self.idxall[:, t, k:k + 1], axis=0),
                )
            gmeta = gt[:, :, 0:METAB].bitcast(F32)  # [P, K, 84] fp32 view

            vec = pools["work"].tile([P, K, 3], F32, tag="vec")
            nc.vector.tensor_tensor(
                out=vec[:, :, :], in0=gmeta[:, :, 0:3],
                in1=self.qxall[:, t, :].unsqueeze(1).to_broadcast([P, K, 3]),
                op=mybir.AluOpType.subtract)
            v2 = pools["work"].tile([P, K, 3], F32, tag="v2")
            nc.vector.tensor_mul(v2[:, :, :], vec[:, :, :], vec[:, :, :])
            d2 = pools["work"].tile([P, K], F32, tag="d2")
            nc.vector.reduce_sum(out=d2[:, :], in_=v2[:, :, :], axis=X)
            dist = pools["work"].tile([P, K], F32, tag="dist")
            nc.scalar.activation(out=dist[:, :], in_=d2[:, :], func=Sqrt)
            nc.vector.tensor_scalar_add(dist[:, :], dist[:, :], EPS_DIR)
            riv = pools["work"].tile([P, K], F32, tag="riv")
            nc.vector.reciprocal(riv[:, :], dist[:, :])
            vecn = pools["work"].tile([P, K, 3], F32, tag="vecn")
            nc.vector.tensor_mul(vecn[:, :, :], vec[:, :, :],
                                 riv[:, :].unsqueeze(2).to_broadcast([P, K, 3]))

            prod = pools["work"].tile([P, K, M, 3], F32, tag="prod")
            nc.vector.tensor_mul(
                prod[:, :, :, :],
                gmeta[:, :, 23:83].rearrange("p k (m c) -> p k m c", c=3),
                vecn[:, :, :].unsqueeze(2).to_broadcast([P, K, M, 3]),
            )
            simm = pools["work"].tile([P, K, M], F32, tag="simm")
            nc.vector.reduce_sum(out=simm[:, :, :], in_=prod[:, :, :, :], axis=X)
            absm = pools["work"].tile([P, K, M], F32, tag="absm")
            nc.scalar.activation(out=absm[:, :, :], in_=simm[:, :, :], func=Abs)
            mask = pools["work"].tile([P, K, M], F32, tag="mask")
            nc.vector.tensor_scalar(out=mask[:, :, :], in0=absm[:, :, :],
                                    scalar1=GAMMA, scalar2=None,
                                    op0=mybir.AluOpType.is_gt)
            mw = pools["work"].tile([P, K, M], F32, tag="mw")
            nc.vector.tensor_mul(mw[:, :, :], mask[:, :, :], gmeta[:, :, 3:23])
            dkw = pools["work"].tile([P, K], F32, tag="dkw")
            nc.vector.reduce_sum(out=dkw[:, :], in_=mw[:, :, :], axis=X)

            dkws = pools["work"].tile([P, 1], F32, tag="dkws")
            nc.vector.reduce_sum(out=dkws[:, :], in_=dkw[:, :], axis=X)
            nc.vector.tensor_scalar_add(dkws[:, :], dkws[:, :], 1e-8)
            r1 = pools["work"].tile([P, 1], F32, tag="r1")
            nc.vector.reciprocal(r1[:, :], dkws[:, :])
            wn = pools["work"].tile([P, K], F32, tag="wn")
            nc.vector.tensor_scalar(out=wn[:, :], in0=dkw[:, :], scalar1=r1[:, 0:1],
                                    scalar2=1e-6, op0=mybir.AluOpType.mult,
                                    op1=mybir.AluOpType.add)
            nc.vector.tensor_scalar_add(wn[:, :], wn[:, :], 1e-10)
            nr2 = pools["work"].tile([P, 1], F32, tag="nr2")
            nc.vector.reduce_sum(out=nr2[:, :], in_=wn[:, :], axis=X)
            nc.vector.tensor_scalar_add(nr2[:, :], nr2[:, :], 1e-8)
            r2 = pools["work"].tile([P, 1], F32, tag="r2")
            nc.vector.reciprocal(r2[:, :], nr2[:, :])
            wp = pools["work"].tile([P, K], F32, tag="wp")
            nc.vector.tensor_scalar(out=wp[:, :], in0=wn[:, :], scalar1=r2[:, 0:1],
                                    scalar2=None, op0=mybir.AluOpType.mult)
            nc.vector.tensor_scalar(out=wp[:, :], in0=wp[:, :], scalar1=dkws[:, 0:1],
                                    scalar2=None, op0=mybir.AluOpType.mult)

            par = pools["work"].tile([P, 1], F32, tag="par")
            nc.gpsimd.partition_all_reduce(par[:, :], dkws[:, :], channels=P,
                                           reduce_op=bass_isa.ReduceOp.add)
            nc.vector.tensor_add(self.acc[:, :], self.acc[:, :], par[:, :])

            dW = pools["work"].tile([P, K, P], BF16, tag="dW")
            for k in range(K):
                nc.scalar.activation(out=dW[:, k, :], in_=self.ident[:, :],
                                     func=Copy, scale=wp[:, k:k + 1])
            po = pools["po"].tile([P, D], F32, tag="po")
            for k in range(K):
                for c0, c1 in ((0, 512), (512, D)):
                    nc.tensor.matmul(out=po[:, c0:c1], lhsT=dW[:, k, :],
                                     rhs=gt[:, k, METAB + c0:METAB + c1],
                                     start=(k == 0), stop=(k == K - 1))
            if self.f1keep is not None:
                nc.scalar.activation(out=self.f1keep[:, t, :], in_=po[:, :], func=Copy)
            else:
                f1 = pools["f1"].tile([P, D], F32, tag="f1")
                nc.scalar.activation(out=f1[:, :], in_=po[:, :], func=Copy)
                nc.sync.dma_start(out=self.f1da[t * P:(t + 1) * P, :], in_=f1[:, :])

    def emit_allreduce(self):
        nc, pools = self.nc, self.pools
        nc.sync.dma_start(out=self.sum_in.ap()[:, :], in_=self.acc[0:1, 0:1])
        nc.gpsimd.collective_compute(
            "AllReduce", mybir.AluOpType.add, replica_groups=RG,
            ins=[self.sum_in.ap()], outs=[self.sum_out.ap()],
        )
        sg = pools["tbl"].tile([P, 1], F32, tag=f"sg_{self.st}")
        nc.sync.dma_start(out=sg[0:1, :], in_=self.sum_out.ap()[:, :])
        sgb = pools["tbl"].tile([P, 1], F32, tag=f"sgb_{self.st}")
        nc.gpsimd.partition_broadcast(sgb[:, :], sg[0:1, :], channels=P)
        scal = pools["tbl"].tile([P, 1], F32, tag=f"scal_{self.st}")
        nc.vector.tensor_scalar(out=scal[:, :], in0=sgb[:, :],
                                scalar1=C_SCAL / self.NT, scalar2=1e-8,
                                op0=mybir.AluOpType.mult, op1=mybir.AluOpType.add)
        self.scal = scal

    def emit_deferred(self):
        """normalize(f1 + scal * p1) -> out rows."""
        nc, pools = self.nc, self.pools
        for t in range(self.n_qt):
            rs = slice(t * P, (t + 1) * P)
            if self.f1keep is not None:
                f1 = self.f1keep[:, t, :]
            else:
                f1t = pools["f1"].tile([P, D], F32, tag="f1b")
                nc.sync.dma_start(out=f1t[:, :], in_=self.f1da[rs, :])
                f1 = f1t[:, :]
            p1t = pools["f1"].tile([P, D], F32, tag="p1t")
            nc.sync.dma_start(out=p1t[:, :], in_=self.p1a[rs, :])
            f2 = pools["f1"].tile([P, D], F32, tag="f2")
            nc.scalar.activation(out=f2[:, :], in_=p1t[:, :], func=Copy,
                                 scale=self.scal[:, 0:1])
            o = pools["f1"].tile([P, D], F32, tag="o")
            nc.vector.tensor_add(o[:, :], f1[:, :], f2[:, :])
            junk = pools["f1"].tile([P, D], F32, tag="junk")
            ss = pools["work"].tile([P, 1], F32, tag="ss")
            nc.scalar.activation(out=junk[:, :], in_=o[:, :], func=Square,
                                 accum_out=ss[:, :])
            nn = pools["work"].tile([P, 1], F32, tag="nn")
            nc.scalar.activation(out=nn[:, :], in_=ss[:, :], func=Sqrt)
            nc.vector.tensor_scalar_max(nn[:, :], nn[:, :], 1e-12)
            ri = pools["work"].tile([P, 1], F32, tag="ri")
            nc.vector.reciprocal(ri[:, :], nn[:, :])
            res = pools["f1"].tile([P, D], BF16 if self.out_bf else F32, tag="res")
            nc.scalar.activation(out=res[:, :], in_=o[:, :], func=Copy,
                                 scale=ri[:, 0:1])
            nc.sync.dma_start(out=self.ora[rs, :], in_=res[:, :])


def build():
    if "nc" in _CACHE:
        return _CACHE["nc"]
    nc = bacc.Bacc("TRN2", num_devices=NCORES)

    tp0 = nc.dram_tensor("tp0", [ST0["S"], TW], F32, kind="ExternalInput")
    tp1 = nc.dram_tensor("tp1", [ST1["S"], 84], F32, kind="ExternalInput")
    q0 = nc.dram_tensor("q0", [ST0["Q"], 3], F32, kind="ExternalInput")
    q1 = nc.dram_tensor("q1", [ST1["Q"], 3], F32, kind="ExternalInput")
    p10 = nc.dram_tensor("p10", [ST0["Q"], D], F32, kind="ExternalInput")
    p11 = nc.dram_tensor("p11", [ST1["Q"], D], F32, kind="ExternalInput")

    out1 = nc.dram_tensor("out1", [ST1["Q"], D], F32, kind="ExternalOutput")

    tw0 = nc.dram_tensor("tw0", [ST0["S"], TWB], BF16)
    tw1 = nc.dram_tensor("tw1", [ST1["S"], TWB], BF16)
    p2s = nc.dram_tensor("p2s", [ST0["Q"], D], BF16)
    p2full = nc.dram_tensor("p2full", [ST1["S"], D], BF16, addr_space="Shared")
    s0in = nc.dram_tensor("s0in", [1, 1], F32)
    s0out = nc.dram_tensor("s0out", [1, 1], F32, addr_space="Shared")
    s1in = nc.dram_tensor("s1in", [1, 1], F32)
    s1out = nc.dram_tensor("s1out", [1, 1], F32, addr_space="Shared")
    f1d0 = nc.dram_tensor("f1d0", [ST0["Q"], D], F32)
    f1d1 = nc.dram_tensor("f1d1", [ST1["Q"], D], F32)

    with TileContext(nc) as tc:
        import contextlib
        with contextlib.ExitStack() as ctx:
            pools = {
                "const": ctx.enter_context(tc.tile_pool(name="const", bufs=1)),
                "tbl": ctx.enter_context(tc.tile_pool(name="tbl", bufs=1)),
                "work": ctx.enter_context(tc.tile_pool(name="work", bufs=2)),
                "neg": ctx.enter_context(tc.tile_pool(name="neg", bufs=3)),
                "gt": ctx.enter_context(tc.tile_pool(name="gt", bufs=3)),
                "f1": ctx.enter_context(tc.tile_pool(name="f1", bufs=2)),
                "pt": ctx.enter_context(tc.tile_pool(name="pt", bufs=1, space="PSUM")),
                "pe": ctx.enter_context(tc.tile_pool(name="pe", bufs=3, space="PSUM")),
                "po": ctx.enter_context(tc.tile_pool(name="po", bufs=2, space="PSUM")),
            }
            ident = pools["const"].tile([P, P], F32, tag="ident")
            make_identity(nc, ident[:, :])

            s0 = Stage(nc, pools, ident, st=0, S=ST0["S"], Q=ST0["Q"],
                       NT=ST0["NT"], tp_src=tp0, tw=tw0, qxyz=q0, p1=p10,
                       f1d=f1d0, out_rows=p2s, sum_in=s0in, sum_out=s0out,
                       fill_feat=True, out_bf=True)
            s1 = Stage(nc, pools, ident, st=1, S=ST1["S"], Q=ST1["Q"],
                       NT=ST1["NT"], tp_src=tp1, tw=tw1, qxyz=q1, p1=p11,
                       f1d=f1d1, out_rows=out1, sum_in=s1in, sum_out=s1out,
                       fill_feat=False, out_bf=False)

            s0.emit_tables()
            s1.emit_tables()
            # stage 0 completely (it gates the AllGather)
            s0.emit_part_a(range(s0.n_qt))
            s0.emit_part_b(range(s0.n_qt))
            s0.emit_allreduce()
            # a few stage-1 ranking tiles to cover the AllReduce latency
            s1.emit_part_a(range(0, 4))
            s0.emit_deferred()
            nc.gpsimd.collective_compute(
                "AllGather", mybir.AluOpType.bypass, replica_groups=RG,
                ins=[p2s.ap()], outs=[p2full.ap()],
            )
            nc.sync.dma_start(out=tw1.ap()[:, METAB:TWB], in_=p2full.ap()[:, :])
            # rest of stage-1 ranking, block-interleaved with the
            # gather/interp pipeline so the PE-bound and GpSimd-bound
            # phases overlap instead of queueing back-to-back
            s1.emit_part_a(range(4, 8))
            s1.emit_part_b(range(0, 2))
            s1.emit_part_a(range(8, 12))
            s1.emit_part_b(range(2, 6))
            s1.emit_part_a(range(12, 16))
            s1.emit_part_b(range(6, 16))
            s1.emit_allreduce()
            s1.emit_deferred()

    nc.compile()
    _CACHE["nc"] = nc
    return nc


def _pack(inputs):
    xyz_c = np.ascontiguousarray(inputs["xyz_c"][0], dtype=np.float32)
    xyz_m = np.ascontiguousarray(inputs["xyz_m"][0], dtype=np.float32)
    xyz_f = np.ascontiguousarray(inputs["xyz_f"][0], dtype=np.float32)
    x_c = np.ascontiguousarray(inputs["x_c"][0], dtype=np.float32)
    x_m = np.ascontiguousarray(inputs["x_m"][0], dtype=np.float32)
    x_f = np.ascontiguousarray(inputs["x_f"][0], dtype=np.float32)
    perc_c = np.ascontiguousarray(inputs["perc_c"][0], dtype=np.float32)
    dir_c = np.ascontiguousarray(inputs["dir_c"][0], dtype=np.float32)
    perc_m = np.ascontiguousarray(inputs["perc_m"][0], dtype=np.float32)
    dir_m = np.ascontiguousarray(inputs["dir_m"][0], dtype=np.float32)

    tp0 = np.zeros((ST0["S"], TW), np.float32)
    tp0[:, 0:3] = xyz_c
    tp0[:, 3:23] = perc_c
    tp0[:, 23:83] = dir_c.reshape(ST0["S"], 60)
    tp0[:, 84:84 + D] = x_c

    tp1 = np.zeros((ST1["S"], 84), np.float32)
    tp1[:, 0:3] = xyz_m
    tp1[:, 3:23] = perc_m
    tp1[:, 23:83] = dir_m.reshape(ST1["S"], 60)

    in_maps = []
    for c in range(NCORES):
        r0 = slice(c * ST0["Q"], (c + 1) * ST0["Q"])
        r1 = slice(c * ST1["Q"], (c + 1) * ST1["Q"])
        in_maps.append({
            "tp0": tp0,
            "tp1": tp1,
            "q0": np.ascontiguousarray(xyz_m[r0]),
            "q1": np.ascontiguousarray(xyz_f[r1]),
            "p10": np.ascontiguousarray(x_m[r0]),
            "p11": np.ascontiguousarray(x_f[r1]),
        })
    return in_maps


def run_sharded(inputs, trace=False, tmpdir=None):
    """Build + run; returns (full_output, BassKernelResults)."""
    from concourse.bass_utils import run_bass_kernel_spmd
    nc = build()
    in_maps = _pack(inputs)
    res = run_bass_kernel_spmd(nc, in_maps, list(range(NCORES)), trace=trace,
                               tmpdir=tmpdir)
    out = np.concatenate([res.results[c]["out1"] for c in range(NCORES)], axis=0)
    return out.reshape(1, ST1["NT"], D).astype(np.float32), res


def kernel(**inputs) -> np.ndarray:
    out, _ = run_sharded(inputs, trace=False)
    return out
